# revision 27
# baseline (speedup 1.0000x reference)
"""Trainium2 Bass kernel for FC+GRU+per-cultivar-head model.

Model (reference):
  x  = input @ W1.T + b1           [B,T,256]
  x  = x @ W2.T + b2               [B,T,512]
  h  = GRU(x, h0; W_ih, W_hh, ...) [B,512]   (PyTorch gate order r,z,n)
  o  = relu(relu(h) @ W3.T + b3)   [B,256]
  p  = einsum('bd,bod->bo', o, Wh[cult]) + bh[cult]   [B,16]
  returns (p, h[None])

Key algebraic fold (host side): the two leading Linears and the GRU input
projection are all linear, so
  gx_t = W_ih @ (W2 @ (W1 @ x_t + b1) + b2) + b_ih = Wg @ x_t + bg
with Wg = W_ih@W2@W1 [1536,32], bg = W_ih@(W2@b1+b2)+b_ih.

Device kernel (per core, batch slice of 128 = free dim, feature-major):
  per step t: PSUM[g,b] += Wg@x_t (K=32) and += W_hh@h (K=4x128), fused;
  r/z accumulate together, xn/hn kept separate (n = tanh(xn + r*hn)).
  Gate math on ACT/DVE/POOL in fp32, h state kept in bf16.
"""
import os
import numpy as np
import ml_dtypes

B, T, D_IN, H, D2, D_OUT, N_CULT = 1024, 128, 32, 512, 256, 16, 32
NCORES = 8
BC = B // NCORES  # batch per core = 128
G3 = 3 * H        # 1536
NM = G3 // 128    # 12 m-tiles
NK = H // 128     # 4 h chunks

BF16 = ml_dtypes.bfloat16

_CACHE = {}


def _build_nc():
    from contextlib import ExitStack

    import concourse.bass as bass
    import concourse.tile as tile
    from concourse import bacc, mybir
    from concourse.bass import ds, ts
    from concourse.masks import make_identity
    from concourse.tile_rust import add_dep_helper

    f32 = mybir.dt.float32
    bf16 = mybir.dt.bfloat16
    AF = mybir.ActivationFunctionType

    nc = bacc.Bacc("TRN2", target_bir_lowering=False, debug=False)

    d_inputT = nc.dram_tensor("inputT", [128, T * BC], bf16, kind="ExternalInput")
    d_wgT = nc.dram_tensor("wgT", [128, G3], bf16, kind="ExternalInput")
    d_whhT = nc.dram_tensor("whhT", [128, NK * G3], bf16, kind="ExternalInput")
    d_w3T = nc.dram_tensor("w3T", [128, NK * D2], bf16, kind="ExternalInput")
    d_whT = nc.dram_tensor("whT", [128, 2 * 512], bf16, kind="ExternalInput")
    d_selT = nc.dram_tensor("selT", [128, 4 * D_OUT], f32, kind="ExternalInput")
    d_hnT = nc.dram_tensor("hnT", [128, H], f32, kind="ExternalInput")
    d_consts = nc.dram_tensor("consts", [128, 26], f32, kind="ExternalInput")
    d_cultf = nc.dram_tensor("cultf", [1, BC], f32, kind="ExternalInput")

    d_out_hn = nc.dram_tensor("out_hn", [BC, H], f32, kind="ExternalOutput")
    d_out_p = nc.dram_tensor("out_p", [BC, D_OUT], f32, kind="ExternalOutput")

    with tile.TileContext(nc) as tc, ExitStack() as ctx:
        wpool = ctx.enter_context(tc.tile_pool(name="wpool", bufs=1))
        state = ctx.enter_context(tc.tile_pool(name="state", bufs=1))
        tmp = ctx.enter_context(tc.tile_pool(name="tmp", bufs=4))

        # ---- load constants/weights ----
        inputT = wpool.tile([128, T * BC], bf16)
        nc.sync.dma_start(inputT[:], d_inputT[:, :])
        wgT = wpool.tile([128, G3], bf16)
        nc.sync.dma_start(wgT[:], d_wgT[:, :])
        whhT = wpool.tile([128, NK * G3], bf16)
        nc.sync.dma_start(whhT[:], d_whhT[:, :])
        w3T = wpool.tile([128, NK * D2], bf16)
        nc.sync.dma_start(w3T[:], d_w3T[:, :])
        whT = wpool.tile([128, 2 * 512], bf16)
        nc.sync.dma_start(whT[:], d_whT[:, :])
        selT = wpool.tile([128, 4 * D_OUT], f32)
        nc.sync.dma_start(selT[:], d_selT[:, :])
        consts = wpool.tile([128, 26], f32)
        nc.sync.dma_start(consts[:], d_consts[:, :])
        cultb = wpool.tile([128, BC], f32)
        nc.gpsimd.dma_start(
            out=cultb[:],
            in_=bass.AP(tensor=d_cultf, offset=0, ap=[[0, 128], [1, BC]]),
        )

        # h state, bf16, feature-major: one tile [128, 4*BC], slice k = h rows
        hpool = ctx.enter_context(tc.tile_pool(name="hpool", bufs=2))
        h_all = hpool.tile([128, NK * BC], bf16, tag="h", name="h_init")
        hnT0 = state.tile([128, H], f32)
        nc.sync.dma_start(hnT0[:], d_hnT[:, :])
        nc.vector.tensor_copy(h_all[:], hnT0[:])

        # b_hh n-part bias, added to ps_hn during eviction (skipped if zero
        # via the HAS_BHN build flag; consts cols 12:16 hold it per chunk).
        C_BHN, C_B3, C_BH, C_IOTA = 12, 16, 18, 22
        HAS_BHN = bool(_CACHE.get("has_bhn", False))

        # ---- the recurrence ----
        # wgT column groups: r = m 0..3, z = m 4..7, n(x) = m 8..11
        # Half-split (A = chunks 0,1; B = chunks 2,3) with one PSUM bank per
        # half and bufs=1: fine-grained deps let each half's gate chain start
        # as soon as its own bank completes. HW constraint: accumulation
        # groups must be contiguous in the PE stream.
        psum_ctx = ExitStack()
        psum = psum_ctx.enter_context(tc.tile_pool(name="psum", bufs=1, space="PSUM"))
        T_STEPS = int(os.environ.get("KERNEL_T", str(T)))
        halves = ((0, 1), (2, 3))
        ps_warm = psum.tile([128, 4 * BC], f32, tag="warm", name="ps_warm")

        for t in range(T_STEPS):
            ps_xn = psum.tile([128, 4 * BC], f32, tag="xn", name=f"ps_xn_{t}")
            for j in range(4):
                nc.tensor.matmul(ps_xn[:, ts(j, BC)], wgT[:, ts(8 + j, 128)],
                                 inputT[:, ts(t, BC)], start=True, stop=True)
            ps_hn = [psum.tile([128, 2 * BC], f32, tag=f"hn{x}", name=f"ps_hn{x}_{t}")
                     for x in range(2)]
            ps_r = [psum.tile([128, 2 * BC], f32, tag=f"r{x}", name=f"ps_r{x}_{t}")
                    for x in range(2)]
            ps_z = [psum.tile([128, 2 * BC], f32, tag=f"z{x}", name=f"ps_z{x}_{t}")
                    for x in range(2)]
            rhs_x = inputT[:, ts(t, BC)]

            def x_mm(ps, sl, gate_m, stop=False):
                nc.tensor.matmul(ps[:, ts(sl, BC)], wgT[:, ts(gate_m, 128)],
                                 rhs_x, start=True, stop=stop)

            def h_mm(ps, sl, gate_m, k, start=False):
                nc.tensor.matmul(
                    ps[:, ts(sl, BC)],
                    whhT[:, ds(k * G3 + gate_m * 128, 128)],
                    h_all[:, ts(k, BC)],
                    start=start, stop=(k == NK - 1),
                )

            # wave 0 (slice 0 of each bank): k-interleaved across the 6 banks
            # so the first MMs need only h chunk 0, then chunk 1, ... matching
            # the staggered arrival of h' from the previous step. wave 1:
            # contiguous groups, completion-ordered (r, hn, z).
            for x in range(2):
                x_mm(ps_r[x], 0, 2 * x)
                x_mm(ps_z[x], 0, 4 + 2 * x)
            for k in range(NK):
                for x in range(2):
                    h_mm(ps_r[x], 0, 2 * x, k)
                    h_mm(ps_hn[x], 0, 8 + 2 * x, k, start=(k == 0))
                    h_mm(ps_z[x], 0, 4 + 2 * x, k)
            for x in range(2):
                x_mm(ps_r[x], 1, 2 * x + 1)
                for k in range(NK):
                    h_mm(ps_r[x], 1, 2 * x + 1, k)
            for x in range(2):
                for k in range(NK):
                    h_mm(ps_hn[x], 1, 8 + 2 * x + 1, k, start=(k == 0))
            for x in range(2):
                x_mm(ps_z[x], 1, 4 + 2 * x + 1)
                for k in range(NK):
                    h_mm(ps_z[x], 1, 4 + 2 * x + 1, k)

            # end-of-step h-free MMs into a never-read scratch bank: keep the
            # HAM clock-gate warm through the h' stall (cold MMs run at 1.2
            # instead of 2.4 GHz and cost ~1us/step otherwise).
            for w in range(10):
                nc.tensor.matmul(ps_warm[:, ts(w % 4, BC)], wgT[:, ts(w, 128)],
                                 inputT[:, ts(min(t + 1, T - 1), BC)],
                                 start=True, stop=True)

            # gate math: per-half chains with FORCED static per-engine order
            # (Tile's static schedule otherwise runs half-B sigmoids before
            # tanh_A, stalling the chain on half-B psums).
            h_nxt = hpool.tile([128, NK * BC], bf16, tag="h", name=f"h_{t}")
            act_prev = dve_prev = None

            def act_chain(bi):
                nonlocal act_prev
                if act_prev is not None:
                    add_dep_helper(bi.ins, act_prev.ins, sync=False,
                                   reason="gate-chain ACT order")
                act_prev = bi

            def dve_chain(bi):
                return bi

            for x, js in enumerate(halves):
                hs = ds(2 * x * BC, 2 * BC)
                sr = tmp.tile([128, 2 * BC], f32, tag=f"sr{x}", name=f"sr{x}_{t}")
                t1 = tmp.tile([128, 2 * BC], f32, tag=f"t1{x}", name=f"t1{x}_{t}")
                n_t = tmp.tile([128, 2 * BC], f32, tag=f"n{x}", name=f"n{x}_{t}")
                t3 = tmp.tile([128, 2 * BC], f32, tag=f"t3{x}", name=f"t3{x}_{t}")
                szc = tmp.tile([128, 2 * BC], f32, tag=f"szc{x}", name=f"szc{x}_{t}")
                t4 = tmp.tile([128, 2 * BC], f32, tag=f"t4{x}", name=f"t4{x}_{t}")
                act_chain(nc.scalar.activation(sr[:], ps_r[x][:], AF.Sigmoid))
                if HAS_BHN:
                    for i, j in enumerate(js):
                        dve_chain(nc.vector.tensor_scalar_add(
                            ps_hn[x][:, ts(i, BC)], ps_hn[x][:, ts(i, BC)],
                            consts[:, C_BHN + j : C_BHN + j + 1]))
                dve_chain(nc.vector.tensor_mul(t1[:], sr[:], ps_hn[x][:]))
                dve_chain(nc.vector.tensor_add(
                    t1[:], t1[:], ps_xn[:, ds(2 * x * BC, 2 * BC)]))
                act_chain(nc.scalar.activation(
                    szc[:], ps_z[x][:], AF.Sigmoid, scale=-1.0))
                act_chain(nc.scalar.activation(n_t[:], t1[:], AF.Tanh))
                # h' = h + (1-z)*(n - h)
                dve_chain(nc.vector.tensor_sub(t3[:], n_t[:], h_all[:, hs]))
                dve_chain(nc.vector.tensor_mul(t4[:], szc[:], t3[:]))
                dve_chain(nc.vector.tensor_add(h_nxt[:, hs], h_all[:, hs], t4[:]))
            h_all = h_nxt

        # ---- epilogue ----
        psum_ctx.close()
        epsum = ctx.enter_context(tc.tile_pool(name="epsum", bufs=1, space="PSUM"))
        ident16 = wpool.tile([128, 128], bf16)
        make_identity(nc, ident16[:])
        ident32 = wpool.tile([128, 128], f32)
        make_identity(nc, ident32[:])

        # hn_out = h_last.T  (batch-major)
        ps_tr = epsum.tile([128, H], bf16)
        for k in range(NK):
            nc.tensor.transpose(ps_tr[:, ts(k, 128)], h_all[:, ts(k, BC)], ident16[:])
        hn_sb = state.tile([128, H], f32)
        nc.vector.tensor_copy(hn_sb[:], ps_tr[:])
        nc.sync.dma_start(d_out_hn[:, :], hn_sb[:])

        # rh = relu(h), bf16, feature-major
        rh = state.tile([128, NK * BC], bf16)
        nc.scalar.activation(rh[:], h_all[:], AF.Relu)

        # out2 = relu(W3 @ rh + b3): [256 -> 2 tiles, BC]
        ps_o2 = epsum.tile([128, 2 * BC], f32)
        for mt in range(2):
            for k in range(NK):
                nc.tensor.matmul(
                    ps_o2[:, ts(mt, BC)],
                    w3T[:, ds(k * D2 + mt * 128, 128)],
                    rh[:, ts(k, BC)],
                    start=(k == 0),
                    stop=(k == NK - 1),
                )
        out2 = state.tile([128, 2 * BC], bf16)
        for mt in range(2):
            nc.scalar.activation(
                out2[:, ts(mt, BC)], ps_o2[:, ts(mt, BC)], AF.Relu,
                bias=consts[:, C_B3 + mt : C_B3 + mt + 1],
            )

        # P_all = Wh_flat @ out2 + bh: [512 (c,o) -> 4 tiles, BC]
        ps_ph = epsum.tile([128, 4 * BC], f32)
        for mt in range(4):
            for k in range(2):
                nc.tensor.matmul(
                    ps_ph[:, ts(mt, BC)],
                    whT[:, ds(k * 512 + mt * 128, 128)],
                    out2[:, ts(k, BC)],
                    start=(k == 0),
                    stop=(k == 1),
                )
        pall = state.tile([128, 4 * BC], f32)
        mask = state.tile([128, BC], f32)
        for mt in range(4):
            nc.vector.tensor_scalar_add(
                pall[:, ts(mt, BC)], ps_ph[:, ts(mt, BC)],
                consts[:, C_BH + mt : C_BH + mt + 1],
            )
            # mask[p, b] = (cult[b] == c_of_row_p)
            nc.vector.tensor_scalar(
                mask[:], cultb[:],
                consts[:, C_IOTA + mt : C_IOTA + mt + 1], None,
                op0=mybir.AluOpType.is_equal,
            )
            nc.vector.tensor_mul(pall[:, ts(mt, BC)], pall[:, ts(mt, BC)], mask[:])

        # params[o, b] = sum_c pall[(c,o), b]  via selector matmul
        ps_pp = epsum.tile([128, BC], f32)
        for k in range(4):
            nc.tensor.matmul(
                ps_pp[0:D_OUT, :], selT[:, ts(k, D_OUT)], pall[:, ts(k, BC)],
                start=(k == 0), stop=(k == 3),
            )
        pp_sb = state.tile([D_OUT, BC], f32)
        nc.vector.tensor_copy(pp_sb[:], ps_pp[0:D_OUT, :])
        # transpose to [BC, 16]
        ps_ptr = epsum.tile([128, D_OUT], f32)
        nc.tensor.transpose(ps_ptr[:, :], pp_sb[:], ident32[0:D_OUT, 0:D_OUT])
        p_sb = state.tile([BC, D_OUT], f32)
        nc.vector.tensor_copy(p_sb[:], ps_ptr[:, :])
        nc.sync.dma_start(d_out_p[:, :], p_sb[:])

    nc.compile()
    return nc


def _prep_host(inputs):
    """Fold weights and build per-core input maps."""
    inp = np.asarray(inputs["input"], np.float32)
    hn = np.asarray(inputs["hn"], np.float32)
    cult = np.asarray(inputs["cultivars"], np.int32)
    W1 = np.asarray(inputs["W1"], np.float64)
    b1 = np.asarray(inputs["b1"], np.float64)
    W2 = np.asarray(inputs["W2"], np.float64)
    b2 = np.asarray(inputs["b2"], np.float64)
    W_ih = np.asarray(inputs["W_ih"], np.float64)
    W_hh = np.asarray(inputs["W_hh"], np.float32)
    b_ih = np.asarray(inputs["b_ih"], np.float64)
    b_hh = np.asarray(inputs["b_hh"], np.float32)
    W3 = np.asarray(inputs["W3"], np.float32)
    b3 = np.asarray(inputs["b3"], np.float32)
    Wh = np.asarray(inputs["Wh"], np.float32)
    bh = np.asarray(inputs["bh"], np.float32)

    Wg = (W_ih @ (W2 @ W1)).astype(np.float32)          # [1536, 32]
    bg = (W_ih @ (W2 @ b1 + b2) + b_ih).astype(np.float32)  # [1536]

    def fm(x, nk):  # [K, M] -> sbuf [128, nk*M] feature-major K-chunked
        K, M = x.shape
        return np.ascontiguousarray(
            x.reshape(nk, 128, M).transpose(1, 0, 2).reshape(128, nk * M)
        )

    bias_row = np.concatenate([
        bg[:1024] + b_hh[:1024].astype(np.float32),   # r,z biases
        bg[1024:],                                    # xn bias
    ])[None, :]                                       # [1, 1536]
    wgT = np.ascontiguousarray(np.concatenate(
        [Wg.T, bias_row, np.zeros((128 - D_IN - 1, G3), np.float32)], axis=0
    )).astype(BF16)                                   # [128, 1536] zero-padded
    whhT = fm(W_hh.T.astype(np.float32), NK).astype(BF16)              # [128, 4*1536]
    w3T = fm(W3.T, NK).astype(BF16)                                    # [128, 4*256]
    Wh_flat = Wh.reshape(N_CULT * D_OUT, D2)                           # [512, 256]
    whT = fm(np.ascontiguousarray(Wh_flat.T), 2).astype(BF16)          # [128, 2*512]
    sel = (np.arange(512)[:, None] % 16 == np.arange(16)[None, :]).astype(np.float32)
    selT = fm(sel, 4)                                                  # [128, 64]

    consts = np.zeros((128, 26), np.float32)
    brz = bg[:1024] + b_hh[:1024].astype(np.float32)
    consts[:, 0:4] = brz[:512].reshape(4, 128).T
    consts[:, 4:8] = -brz[512:1024].reshape(4, 128).T
    consts[:, 8:12] = bg[1024:].reshape(4, 128).T
    consts[:, 12:16] = b_hh[1024:].reshape(4, 128).T
    consts[:, 16:18] = b3.reshape(2, 128).T
    consts[:, 18:22] = bh.reshape(512)[:, None].reshape(4, 128).T
    consts[:, 22:26] = ((np.arange(512) // 16).astype(np.float32)).reshape(4, 128).T

    shared = {
        "wgT": wgT, "whhT": whhT, "w3T": w3T, "whT": whT,
        "selT": selT, "consts": consts,
    }

    in_maps = []
    for c in range(NCORES):
        b0 = c * BC
        ic = inp[b0 : b0 + BC]                       # [BC, T, D]
        inputT = np.ascontiguousarray(np.concatenate([
            ic.transpose(2, 1, 0).reshape(D_IN, T * BC),
            np.ones((1, T * BC), np.float32),
            np.zeros((128 - D_IN - 1, T * BC), np.float32),
        ], axis=0)).astype(BF16)                      # [128, T*BC], free=(t,b)
        h0 = hn[0, b0 : b0 + BC]                      # [BC, H]
        hnT = np.ascontiguousarray(
            h0.T.reshape(NK, 128, BC).transpose(1, 0, 2).reshape(128, H)
        )
        cultf = cult[b0 : b0 + BC, 0].astype(np.float32)[None, :]
        in_maps.append({**shared, "inputT": inputT, "hnT": hnT, "cultf": cultf})
    return in_maps


def _ensure_ntff_hook():
    """Inject antenv.axon_hooks (absent in this image) so trace=True works."""
    import sys
    import types

    try:
        from antenv.axon_hooks import get_axon_ntff_profile_hook  # noqa: F401
        return
    except ImportError:
        pass
    try:
        import antenv
        from trn_agent_boot.trn_boot import _ntff_profile_via_ctypes

        hook = _ntff_profile_via_ctypes("/opt/axon/libaxon_pjrt.so")
        mod = types.ModuleType("antenv.axon_hooks")
        mod._hook = hook
        mod.set_axon_ntff_profile_hook = lambda h: setattr(mod, "_hook", h)
        mod.get_axon_ntff_profile_hook = lambda: mod._hook
        sys.modules["antenv.axon_hooks"] = mod
        antenv.axon_hooks = mod
    except Exception as e:  # degrade to no-trace
        print(f"ntff hook setup failed: {e}")


def kernel(**inputs):
    from concourse.bass_utils import run_bass_kernel_spmd

    if bool(int(os.environ.get("KERNEL_TRACE", "0"))):
        _ensure_ntff_hook()
    has_bhn = bool(np.any(np.asarray(inputs["b_hh"])[1024:] != 0))
    if _CACHE.get("nc") is None or _CACHE.get("has_bhn") != has_bhn:
        _CACHE["has_bhn"] = has_bhn
        _CACHE["nc"] = _build_nc()
    nc = _CACHE["nc"]

    in_maps = _prep_host(inputs)
    res = run_bass_kernel_spmd(
        nc, in_maps, core_ids=list(range(NCORES)),
        trace=bool(int(os.environ.get("KERNEL_TRACE", "0"))),
    )
    if res.exec_time_ns is not None:
        _CACHE["exec_time_ns"] = res.exec_time_ns
        _CACHE["results"] = res

    params = np.concatenate([r["out_p"] for r in res.results], axis=0)
    hn_out = np.concatenate([r["out_hn"] for r in res.results], axis=0)[None]
    return (params.astype(np.float32), hn_out.astype(np.float32))


# revision 28
# speedup vs baseline: 1.0291x; 1.0291x over previous
"""Trainium2 Bass kernel for FC+GRU+per-cultivar-head model.

Model (reference):
  x  = input @ W1.T + b1           [B,T,256]
  x  = x @ W2.T + b2               [B,T,512]
  h  = GRU(x, h0; W_ih, W_hh, ...) [B,512]   (PyTorch gate order r,z,n)
  o  = relu(relu(h) @ W3.T + b3)   [B,256]
  p  = einsum('bd,bod->bo', o, Wh[cult]) + bh[cult]   [B,16]
  returns (p, h[None])

Key algebraic fold (host side): the two leading Linears and the GRU input
projection are all linear, so
  gx_t = W_ih @ (W2 @ (W1 @ x_t + b1) + b2) + b_ih = Wg @ x_t + bg
with Wg = W_ih@W2@W1 [1536,32], bg = W_ih@(W2@b1+b2)+b_ih.

Device kernel (per core, batch slice of 128 = free dim, feature-major):
  per step t: PSUM[g,b] += Wg@x_t (K=32) and += W_hh@h (K=4x128), fused;
  r/z accumulate together, xn/hn kept separate (n = tanh(xn + r*hn)).
  Gate math on ACT/DVE/POOL in fp32, h state kept in bf16.
"""
import os
import numpy as np
import ml_dtypes

B, T, D_IN, H, D2, D_OUT, N_CULT = 1024, 128, 32, 512, 256, 16, 32
NCORES = 8
BC = B // NCORES  # batch per core = 128
G3 = 3 * H        # 1536
NM = G3 // 128    # 12 m-tiles
NK = H // 128     # 4 h chunks

BF16 = ml_dtypes.bfloat16

_CACHE = {}


def _build_nc():
    from contextlib import ExitStack

    import concourse.bass as bass
    import concourse.tile as tile
    from concourse import bacc, mybir
    from concourse.bass import ds, ts
    from concourse.masks import make_identity
    from concourse.tile_rust import add_dep_helper

    f32 = mybir.dt.float32
    bf16 = mybir.dt.bfloat16
    AF = mybir.ActivationFunctionType

    nc = bacc.Bacc("TRN2", target_bir_lowering=False, debug=False)

    d_inputT = nc.dram_tensor("inputT", [128, T * BC], bf16, kind="ExternalInput")
    d_wgT = nc.dram_tensor("wgT", [128, G3], bf16, kind="ExternalInput")
    d_whhT = nc.dram_tensor("whhT", [128, NK * G3], bf16, kind="ExternalInput")
    d_w3T = nc.dram_tensor("w3T", [128, NK * D2], bf16, kind="ExternalInput")
    d_whT = nc.dram_tensor("whT", [128, 2 * 512], bf16, kind="ExternalInput")
    d_selT = nc.dram_tensor("selT", [128, 4 * D_OUT], f32, kind="ExternalInput")
    d_hnT = nc.dram_tensor("hnT", [128, H], f32, kind="ExternalInput")
    d_consts = nc.dram_tensor("consts", [128, 26], f32, kind="ExternalInput")
    d_cultf = nc.dram_tensor("cultf", [1, BC], f32, kind="ExternalInput")

    d_out_hn = nc.dram_tensor("out_hn", [BC, H], f32, kind="ExternalOutput")
    d_out_p = nc.dram_tensor("out_p", [BC, D_OUT], f32, kind="ExternalOutput")

    with tile.TileContext(nc) as tc, ExitStack() as ctx:
        wpool = ctx.enter_context(tc.tile_pool(name="wpool", bufs=1))
        state = ctx.enter_context(tc.tile_pool(name="state", bufs=1))
        tmp = ctx.enter_context(tc.tile_pool(name="tmp", bufs=4))

        # ---- load constants/weights ----
        inputT = wpool.tile([128, T * BC], bf16)
        nc.sync.dma_start(inputT[:], d_inputT[:, :])
        wgT = wpool.tile([128, G3], bf16)
        nc.sync.dma_start(wgT[:], d_wgT[:, :])
        whhT = wpool.tile([128, NK * G3], bf16)
        nc.sync.dma_start(whhT[:], d_whhT[:, :])
        w3T = wpool.tile([128, NK * D2], bf16)
        nc.sync.dma_start(w3T[:], d_w3T[:, :])
        whT = wpool.tile([128, 2 * 512], bf16)
        nc.sync.dma_start(whT[:], d_whT[:, :])
        selT = wpool.tile([128, 4 * D_OUT], f32)
        nc.sync.dma_start(selT[:], d_selT[:, :])
        consts = wpool.tile([128, 26], f32)
        nc.sync.dma_start(consts[:], d_consts[:, :])
        cultb = wpool.tile([128, BC], f32)
        nc.gpsimd.dma_start(
            out=cultb[:],
            in_=bass.AP(tensor=d_cultf, offset=0, ap=[[0, 128], [1, BC]]),
        )

        # h state, bf16, feature-major: one tile [128, 4*BC], slice k = h rows
        hpool = ctx.enter_context(tc.tile_pool(name="hpool", bufs=2))
        h_all = hpool.tile([128, NK * BC], bf16, tag="h", name="h_init")
        hnT0 = state.tile([128, H], f32)
        nc.sync.dma_start(hnT0[:], d_hnT[:, :])
        nc.vector.tensor_copy(h_all[:], hnT0[:])

        # b_hh n-part bias, added to ps_hn during eviction (skipped if zero
        # via the HAS_BHN build flag; consts cols 12:16 hold it per chunk).
        C_BHN, C_B3, C_BH, C_IOTA = 12, 16, 18, 22
        HAS_BHN = bool(_CACHE.get("has_bhn", False))

        # ---- the recurrence ----
        # wgT column groups: r = m 0..3, z = m 4..7, n(x) = m 8..11
        # Half-split (A = chunks 0,1; B = chunks 2,3) with one PSUM bank per
        # half and bufs=1: fine-grained deps let each half's gate chain start
        # as soon as its own bank completes. HW constraint: accumulation
        # groups must be contiguous in the PE stream.
        psum_ctx = ExitStack()
        psum = psum_ctx.enter_context(tc.tile_pool(name="psum", bufs=1, space="PSUM"))
        T_STEPS = int(os.environ.get("KERNEL_T", str(T)))
        halves = ((0, 1), (2, 3))
        ps_warm = psum.tile([128, 4 * BC], f32, tag="warm", name="ps_warm")

        for t in range(T_STEPS):
            ps_xn = psum.tile([128, 4 * BC], f32, tag="xn", name=f"ps_xn_{t}")
            for j in range(4):
                nc.tensor.matmul(ps_xn[:, ts(j, BC)], wgT[:, ts(8 + j, 128)],
                                 inputT[:, ts(t, BC)], start=True, stop=True)
            ps_hn = [psum.tile([128, 2 * BC], f32, tag=f"hn{x}", name=f"ps_hn{x}_{t}")
                     for x in range(2)]
            ps_r = [psum.tile([128, 2 * BC], f32, tag=f"r{x}", name=f"ps_r{x}_{t}")
                    for x in range(2)]
            ps_z = [psum.tile([128, 2 * BC], f32, tag=f"z{x}", name=f"ps_z{x}_{t}")
                    for x in range(2)]
            rhs_x = inputT[:, ts(t, BC)]

            def x_mm(ps, sl, gate_m, stop=False):
                nc.tensor.matmul(ps[:, ts(sl, BC)], wgT[:, ts(gate_m, 128)],
                                 rhs_x, start=True, stop=stop)

            def h_mm(ps, sl, gate_m, k, start=False):
                nc.tensor.matmul(
                    ps[:, ts(sl, BC)],
                    whhT[:, ds(k * G3 + gate_m * 128, 128)],
                    h_all[:, ts(k, BC)],
                    start=start, stop=(k == NK - 1),
                )

            # wave 0 (slice 0 of each bank): k-interleaved across the 6 banks
            # so the first MMs need only h chunk 0, then chunk 1, ... matching
            # the staggered arrival of h' from the previous step. wave 1:
            # contiguous groups, completion-ordered (r, hn, z).
            for x in range(2):
                x_mm(ps_r[x], 0, 2 * x)
                x_mm(ps_z[x], 0, 4 + 2 * x)
            for k in range(NK):
                for x in range(2):
                    h_mm(ps_r[x], 0, 2 * x, k)
                    h_mm(ps_hn[x], 0, 8 + 2 * x, k, start=(k == 0))
                    h_mm(ps_z[x], 0, 4 + 2 * x, k)
            for x in range(2):
                x_mm(ps_r[x], 1, 2 * x + 1)
                for k in range(NK):
                    h_mm(ps_r[x], 1, 2 * x + 1, k)
            for x in range(2):
                for k in range(NK):
                    h_mm(ps_hn[x], 1, 8 + 2 * x + 1, k, start=(k == 0))
            for x in range(2):
                x_mm(ps_z[x], 1, 4 + 2 * x + 1)
                for k in range(NK):
                    h_mm(ps_z[x], 1, 4 + 2 * x + 1, k)

            # end-of-step h-free MMs into a never-read scratch bank: keep the
            # HAM clock-gate warm through the h' stall (cold MMs run at 1.2
            # instead of 2.4 GHz and cost ~1us/step otherwise).
            for w in range(10):
                nc.tensor.matmul(ps_warm[:, ts(w % 4, BC)], wgT[:, ts(w, 128)],
                                 inputT[:, ts(min(t + 1, T - 1), BC)],
                                 start=True, stop=True)

            # gate math: per-half chains with FORCED static per-engine order
            # (Tile's static schedule otherwise runs half-B sigmoids before
            # tanh_A, stalling the chain on half-B psums).
            h_nxt = hpool.tile([128, NK * BC], bf16, tag="h", name=f"h_{t}")
            act_prev = dve_prev = None

            def act_chain(bi):
                return bi

            def dve_chain(bi):
                return bi

            for x, js in enumerate(halves):
                hs = ds(2 * x * BC, 2 * BC)
                sr = tmp.tile([128, 2 * BC], f32, tag=f"sr{x}", name=f"sr{x}_{t}")
                t1 = tmp.tile([128, 2 * BC], f32, tag=f"t1{x}", name=f"t1{x}_{t}")
                n_t = tmp.tile([128, 2 * BC], f32, tag=f"n{x}", name=f"n{x}_{t}")
                t3 = tmp.tile([128, 2 * BC], f32, tag=f"t3{x}", name=f"t3{x}_{t}")
                szc = tmp.tile([128, 2 * BC], f32, tag=f"szc{x}", name=f"szc{x}_{t}")
                t4 = tmp.tile([128, 2 * BC], f32, tag=f"t4{x}", name=f"t4{x}_{t}")
                act_chain(nc.scalar.activation(sr[:], ps_r[x][:], AF.Sigmoid))
                if HAS_BHN:
                    for i, j in enumerate(js):
                        dve_chain(nc.vector.tensor_scalar_add(
                            ps_hn[x][:, ts(i, BC)], ps_hn[x][:, ts(i, BC)],
                            consts[:, C_BHN + j : C_BHN + j + 1]))
                dve_chain(nc.vector.tensor_mul(t1[:], sr[:], ps_hn[x][:]))
                dve_chain(nc.vector.tensor_add(
                    t1[:], t1[:], ps_xn[:, ds(2 * x * BC, 2 * BC)]))
                act_chain(nc.scalar.activation(
                    szc[:], ps_z[x][:], AF.Sigmoid, scale=-1.0))
                act_chain(nc.scalar.activation(n_t[:], t1[:], AF.Tanh))
                # h' = h + (1-z)*(n - h)
                dve_chain(nc.vector.tensor_sub(t3[:], n_t[:], h_all[:, hs]))
                dve_chain(nc.vector.tensor_mul(t4[:], szc[:], t3[:]))
                dve_chain(nc.vector.tensor_add(h_nxt[:, hs], h_all[:, hs], t4[:]))
            h_all = h_nxt

        # ---- epilogue ----
        psum_ctx.close()
        epsum = ctx.enter_context(tc.tile_pool(name="epsum", bufs=1, space="PSUM"))
        ident16 = wpool.tile([128, 128], bf16)
        make_identity(nc, ident16[:])
        ident32 = wpool.tile([128, 128], f32)
        make_identity(nc, ident32[:])

        # hn_out = h_last.T  (batch-major)
        ps_tr = epsum.tile([128, H], bf16)
        for k in range(NK):
            nc.tensor.transpose(ps_tr[:, ts(k, 128)], h_all[:, ts(k, BC)], ident16[:])
        hn_sb = state.tile([128, H], f32)
        nc.vector.tensor_copy(hn_sb[:], ps_tr[:])
        nc.sync.dma_start(d_out_hn[:, :], hn_sb[:])

        # rh = relu(h), bf16, feature-major
        rh = state.tile([128, NK * BC], bf16)
        nc.scalar.activation(rh[:], h_all[:], AF.Relu)

        # out2 = relu(W3 @ rh + b3): [256 -> 2 tiles, BC]
        ps_o2 = epsum.tile([128, 2 * BC], f32)
        for mt in range(2):
            for k in range(NK):
                nc.tensor.matmul(
                    ps_o2[:, ts(mt, BC)],
                    w3T[:, ds(k * D2 + mt * 128, 128)],
                    rh[:, ts(k, BC)],
                    start=(k == 0),
                    stop=(k == NK - 1),
                )
        out2 = state.tile([128, 2 * BC], bf16)
        for mt in range(2):
            nc.scalar.activation(
                out2[:, ts(mt, BC)], ps_o2[:, ts(mt, BC)], AF.Relu,
                bias=consts[:, C_B3 + mt : C_B3 + mt + 1],
            )

        # P_all = Wh_flat @ out2 + bh: [512 (c,o) -> 4 tiles, BC]
        ps_ph = epsum.tile([128, 4 * BC], f32)
        for mt in range(4):
            for k in range(2):
                nc.tensor.matmul(
                    ps_ph[:, ts(mt, BC)],
                    whT[:, ds(k * 512 + mt * 128, 128)],
                    out2[:, ts(k, BC)],
                    start=(k == 0),
                    stop=(k == 1),
                )
        pall = state.tile([128, 4 * BC], f32)
        mask = state.tile([128, BC], f32)
        for mt in range(4):
            nc.vector.tensor_scalar_add(
                pall[:, ts(mt, BC)], ps_ph[:, ts(mt, BC)],
                consts[:, C_BH + mt : C_BH + mt + 1],
            )
            # mask[p, b] = (cult[b] == c_of_row_p)
            nc.vector.tensor_scalar(
                mask[:], cultb[:],
                consts[:, C_IOTA + mt : C_IOTA + mt + 1], None,
                op0=mybir.AluOpType.is_equal,
            )
            nc.vector.tensor_mul(pall[:, ts(mt, BC)], pall[:, ts(mt, BC)], mask[:])

        # params[o, b] = sum_c pall[(c,o), b]  via selector matmul
        ps_pp = epsum.tile([128, BC], f32)
        for k in range(4):
            nc.tensor.matmul(
                ps_pp[0:D_OUT, :], selT[:, ts(k, D_OUT)], pall[:, ts(k, BC)],
                start=(k == 0), stop=(k == 3),
            )
        pp_sb = state.tile([D_OUT, BC], f32)
        nc.vector.tensor_copy(pp_sb[:], ps_pp[0:D_OUT, :])
        # transpose to [BC, 16]
        ps_ptr = epsum.tile([128, D_OUT], f32)
        nc.tensor.transpose(ps_ptr[:, :], pp_sb[:], ident32[0:D_OUT, 0:D_OUT])
        p_sb = state.tile([BC, D_OUT], f32)
        nc.vector.tensor_copy(p_sb[:], ps_ptr[:, :])
        nc.sync.dma_start(d_out_p[:, :], p_sb[:])

    nc.compile()
    return nc


def _prep_host(inputs):
    """Fold weights and build per-core input maps."""
    inp = np.asarray(inputs["input"], np.float32)
    hn = np.asarray(inputs["hn"], np.float32)
    cult = np.asarray(inputs["cultivars"], np.int32)
    W1 = np.asarray(inputs["W1"], np.float64)
    b1 = np.asarray(inputs["b1"], np.float64)
    W2 = np.asarray(inputs["W2"], np.float64)
    b2 = np.asarray(inputs["b2"], np.float64)
    W_ih = np.asarray(inputs["W_ih"], np.float64)
    W_hh = np.asarray(inputs["W_hh"], np.float32)
    b_ih = np.asarray(inputs["b_ih"], np.float64)
    b_hh = np.asarray(inputs["b_hh"], np.float32)
    W3 = np.asarray(inputs["W3"], np.float32)
    b3 = np.asarray(inputs["b3"], np.float32)
    Wh = np.asarray(inputs["Wh"], np.float32)
    bh = np.asarray(inputs["bh"], np.float32)

    Wg = (W_ih @ (W2 @ W1)).astype(np.float32)          # [1536, 32]
    bg = (W_ih @ (W2 @ b1 + b2) + b_ih).astype(np.float32)  # [1536]

    def fm(x, nk):  # [K, M] -> sbuf [128, nk*M] feature-major K-chunked
        K, M = x.shape
        return np.ascontiguousarray(
            x.reshape(nk, 128, M).transpose(1, 0, 2).reshape(128, nk * M)
        )

    bias_row = np.concatenate([
        bg[:1024] + b_hh[:1024].astype(np.float32),   # r,z biases
        bg[1024:],                                    # xn bias
    ])[None, :]                                       # [1, 1536]
    wgT = np.ascontiguousarray(np.concatenate(
        [Wg.T, bias_row, np.zeros((128 - D_IN - 1, G3), np.float32)], axis=0
    )).astype(BF16)                                   # [128, 1536] zero-padded
    whhT = fm(W_hh.T.astype(np.float32), NK).astype(BF16)              # [128, 4*1536]
    w3T = fm(W3.T, NK).astype(BF16)                                    # [128, 4*256]
    Wh_flat = Wh.reshape(N_CULT * D_OUT, D2)                           # [512, 256]
    whT = fm(np.ascontiguousarray(Wh_flat.T), 2).astype(BF16)          # [128, 2*512]
    sel = (np.arange(512)[:, None] % 16 == np.arange(16)[None, :]).astype(np.float32)
    selT = fm(sel, 4)                                                  # [128, 64]

    consts = np.zeros((128, 26), np.float32)
    brz = bg[:1024] + b_hh[:1024].astype(np.float32)
    consts[:, 0:4] = brz[:512].reshape(4, 128).T
    consts[:, 4:8] = -brz[512:1024].reshape(4, 128).T
    consts[:, 8:12] = bg[1024:].reshape(4, 128).T
    consts[:, 12:16] = b_hh[1024:].reshape(4, 128).T
    consts[:, 16:18] = b3.reshape(2, 128).T
    consts[:, 18:22] = bh.reshape(512)[:, None].reshape(4, 128).T
    consts[:, 22:26] = ((np.arange(512) // 16).astype(np.float32)).reshape(4, 128).T

    shared = {
        "wgT": wgT, "whhT": whhT, "w3T": w3T, "whT": whT,
        "selT": selT, "consts": consts,
    }

    in_maps = []
    for c in range(NCORES):
        b0 = c * BC
        ic = inp[b0 : b0 + BC]                       # [BC, T, D]
        inputT = np.ascontiguousarray(np.concatenate([
            ic.transpose(2, 1, 0).reshape(D_IN, T * BC),
            np.ones((1, T * BC), np.float32),
            np.zeros((128 - D_IN - 1, T * BC), np.float32),
        ], axis=0)).astype(BF16)                      # [128, T*BC], free=(t,b)
        h0 = hn[0, b0 : b0 + BC]                      # [BC, H]
        hnT = np.ascontiguousarray(
            h0.T.reshape(NK, 128, BC).transpose(1, 0, 2).reshape(128, H)
        )
        cultf = cult[b0 : b0 + BC, 0].astype(np.float32)[None, :]
        in_maps.append({**shared, "inputT": inputT, "hnT": hnT, "cultf": cultf})
    return in_maps


def _ensure_ntff_hook():
    """Inject antenv.axon_hooks (absent in this image) so trace=True works."""
    import sys
    import types

    try:
        from antenv.axon_hooks import get_axon_ntff_profile_hook  # noqa: F401
        return
    except ImportError:
        pass
    try:
        import antenv
        from trn_agent_boot.trn_boot import _ntff_profile_via_ctypes

        hook = _ntff_profile_via_ctypes("/opt/axon/libaxon_pjrt.so")
        mod = types.ModuleType("antenv.axon_hooks")
        mod._hook = hook
        mod.set_axon_ntff_profile_hook = lambda h: setattr(mod, "_hook", h)
        mod.get_axon_ntff_profile_hook = lambda: mod._hook
        sys.modules["antenv.axon_hooks"] = mod
        antenv.axon_hooks = mod
    except Exception as e:  # degrade to no-trace
        print(f"ntff hook setup failed: {e}")


def kernel(**inputs):
    from concourse.bass_utils import run_bass_kernel_spmd

    if bool(int(os.environ.get("KERNEL_TRACE", "0"))):
        _ensure_ntff_hook()
    has_bhn = bool(np.any(np.asarray(inputs["b_hh"])[1024:] != 0))
    if _CACHE.get("nc") is None or _CACHE.get("has_bhn") != has_bhn:
        _CACHE["has_bhn"] = has_bhn
        _CACHE["nc"] = _build_nc()
    nc = _CACHE["nc"]

    in_maps = _prep_host(inputs)
    res = run_bass_kernel_spmd(
        nc, in_maps, core_ids=list(range(NCORES)),
        trace=bool(int(os.environ.get("KERNEL_TRACE", "0"))),
    )
    if res.exec_time_ns is not None:
        _CACHE["exec_time_ns"] = res.exec_time_ns
        _CACHE["results"] = res

    params = np.concatenate([r["out_p"] for r in res.results], axis=0)
    hn_out = np.concatenate([r["out_hn"] for r in res.results], axis=0)[None]
    return (params.astype(np.float32), hn_out.astype(np.float32))


# revision 29
# speedup vs baseline: 1.2903x; 1.2539x over previous
"""Trainium2 Bass kernel for FC+GRU+per-cultivar-head model.

Model (reference):
  x  = input @ W1.T + b1           [B,T,256]
  x  = x @ W2.T + b2               [B,T,512]
  h  = GRU(x, h0; W_ih, W_hh, ...) [B,512]   (PyTorch gate order r,z,n)
  o  = relu(relu(h) @ W3.T + b3)   [B,256]
  p  = einsum('bd,bod->bo', o, Wh[cult]) + bh[cult]   [B,16]
  returns (p, h[None])

Key algebraic fold (host side): the two leading Linears and the GRU input
projection are all linear, so
  gx_t = W_ih @ (W2 @ (W1 @ x_t + b1) + b2) + b_ih = Wg @ x_t + bg
with Wg = W_ih@W2@W1 [1536,32], bg = W_ih@(W2@b1+b2)+b_ih.

Device kernel (per core, batch slice of 128 = free dim, feature-major):
  per step t: PSUM[g,b] += Wg@x_t (K=32) and += W_hh@h (K=4x128), fused;
  r/z accumulate together, xn/hn kept separate (n = tanh(xn + r*hn)).
  Gate math on ACT/DVE/POOL in fp32, h state kept in bf16.
"""
import os
import numpy as np
import ml_dtypes

B, T, D_IN, H, D2, D_OUT, N_CULT = 1024, 128, 32, 512, 256, 16, 32
NCORES = 8
BC = B // NCORES  # batch per core = 128
G3 = 3 * H        # 1536
NM = G3 // 128    # 12 m-tiles
NK = H // 128     # 4 h chunks

BF16 = ml_dtypes.bfloat16

_CACHE = {}


def _build_nc():
    from contextlib import ExitStack

    import concourse.bass as bass
    import concourse.tile as tile
    from concourse import bacc, mybir
    from concourse.bass import ds, ts
    from concourse.masks import make_identity
    from concourse.tile_rust import add_dep_helper

    f32 = mybir.dt.float32
    bf16 = mybir.dt.bfloat16
    AF = mybir.ActivationFunctionType

    nc = bacc.Bacc("TRN2", target_bir_lowering=False, debug=False)

    d_inputT = nc.dram_tensor("inputT", [128, T * BC], bf16, kind="ExternalInput")
    d_wgT = nc.dram_tensor("wgT", [128, G3], bf16, kind="ExternalInput")
    d_whhT = nc.dram_tensor("whhT", [128, NK * G3], bf16, kind="ExternalInput")
    d_w3T = nc.dram_tensor("w3T", [128, NK * D2], bf16, kind="ExternalInput")
    d_whT = nc.dram_tensor("whT", [128, 2 * 512], bf16, kind="ExternalInput")
    d_selT = nc.dram_tensor("selT", [128, 4 * D_OUT], f32, kind="ExternalInput")
    d_hnT = nc.dram_tensor("hnT", [128, H], f32, kind="ExternalInput")
    d_consts = nc.dram_tensor("consts", [128, 26], f32, kind="ExternalInput")
    d_cultf = nc.dram_tensor("cultf", [1, BC], f32, kind="ExternalInput")

    d_out_hn = nc.dram_tensor("out_hn", [BC, H], f32, kind="ExternalOutput")
    d_out_p = nc.dram_tensor("out_p", [BC, D_OUT], f32, kind="ExternalOutput")

    with tile.TileContext(nc) as tc, ExitStack() as ctx:
        wpool = ctx.enter_context(tc.tile_pool(name="wpool", bufs=1))
        state = ctx.enter_context(tc.tile_pool(name="state", bufs=1))
        tmp = ctx.enter_context(tc.tile_pool(name="tmp", bufs=4))

        # ---- load constants/weights ----
        inputT = wpool.tile([128, T * BC], bf16)
        nc.sync.dma_start(inputT[:], d_inputT[:, :])
        wgT = wpool.tile([128, G3], bf16)
        nc.sync.dma_start(wgT[:], d_wgT[:, :])
        whhT = wpool.tile([128, NK * G3], bf16)
        nc.sync.dma_start(whhT[:], d_whhT[:, :])
        w3T = wpool.tile([128, NK * D2], bf16)
        nc.sync.dma_start(w3T[:], d_w3T[:, :])
        whT = wpool.tile([128, 2 * 512], bf16)
        nc.sync.dma_start(whT[:], d_whT[:, :])
        selT = wpool.tile([128, 4 * D_OUT], f32)
        nc.sync.dma_start(selT[:], d_selT[:, :])
        consts = wpool.tile([128, 26], f32)
        nc.sync.dma_start(consts[:], d_consts[:, :])
        cultb = wpool.tile([128, BC], f32)
        nc.gpsimd.dma_start(
            out=cultb[:],
            in_=bass.AP(tensor=d_cultf, offset=0, ap=[[0, 128], [1, BC]]),
        )

        # h state, bf16, feature-major: one tile [128, 4*BC], slice k = h rows
        hpool = ctx.enter_context(tc.tile_pool(name="hpool", bufs=2))
        h_all = hpool.tile([128, NK * BC], bf16, tag="h", name="h_init")
        hnT0 = state.tile([128, H], f32)
        nc.sync.dma_start(hnT0[:], d_hnT[:, :])
        nc.vector.tensor_copy(h_all[:], hnT0[:])

        # b_hh n-part bias, added to ps_hn during eviction (skipped if zero
        # via the HAS_BHN build flag; consts cols 12:16 hold it per chunk).
        C_BHN, C_B3, C_BH, C_IOTA = 12, 16, 18, 22
        HAS_BHN = bool(_CACHE.get("has_bhn", False))

        # ---- the recurrence ----
        # wgT column groups: r = m 0..3, z = m 4..7, n(x) = m 8..11
        # Half-split (A = chunks 0,1; B = chunks 2,3) with one PSUM bank per
        # half and bufs=1: fine-grained deps let each half's gate chain start
        # as soon as its own bank completes. HW constraint: accumulation
        # groups must be contiguous in the PE stream.
        psum_ctx = ExitStack()
        psum = psum_ctx.enter_context(tc.tile_pool(name="psum", bufs=1, space="PSUM"))
        T_STEPS = int(os.environ.get("KERNEL_T", str(T)))
        halves = ((0, 1), (2, 3))
        ps_warm = psum.tile([128, 4 * BC], f32, tag="warm", name="ps_warm")

        for t in range(T_STEPS):
            ps_xn = psum.tile([128, 4 * BC], f32, tag="xn", name=f"ps_xn_{t}")
            for j in range(4):
                nc.tensor.matmul(ps_xn[:, ts(j, BC)], wgT[:, ts(8 + j, 128)],
                                 inputT[:, ts(t, BC)], start=True, stop=True)
            ps_hn = [psum.tile([128, 2 * BC], f32, tag=f"hn{x}", name=f"ps_hn{x}_{t}")
                     for x in range(2)]
            ps_r = [psum.tile([128, 2 * BC], f32, tag=f"r{x}", name=f"ps_r{x}_{t}")
                    for x in range(2)]
            ps_z = [psum.tile([128, 2 * BC], f32, tag=f"z{x}", name=f"ps_z{x}_{t}")
                    for x in range(2)]
            rhs_x = inputT[:, ts(t, BC)]

            def x_mm(ps, sl, gate_m, stop=False):
                nc.tensor.matmul(ps[:, ts(sl, BC)], wgT[:, ts(gate_m, 128)],
                                 rhs_x, start=True, stop=stop)

            def h_mm(ps, sl, gate_m, k, start=False):
                nc.tensor.matmul(
                    ps[:, ts(sl, BC)],
                    whhT[:, ds(k * G3 + gate_m * 128, 128)],
                    h_all[:, ts(k, BC)],
                    start=start, stop=(k == NK - 1),
                )

            # wave 0 (slice 0 of each bank): k-interleaved across the 6 banks
            # so the first MMs need only h chunk 0, then chunk 1, ... matching
            # the staggered arrival of h' from the previous step. wave 1:
            # contiguous groups, completion-ordered (r, hn, z).
            for x in range(2):
                x_mm(ps_r[x], 0, 2 * x)
                x_mm(ps_z[x], 0, 4 + 2 * x)
            for k in range(NK):
                for x in range(2):
                    h_mm(ps_r[x], 0, 2 * x, k)
                    h_mm(ps_hn[x], 0, 8 + 2 * x, k, start=(k == 0))
                    h_mm(ps_z[x], 0, 4 + 2 * x, k)
            for x in range(2):
                x_mm(ps_r[x], 1, 2 * x + 1)
                for k in range(NK):
                    h_mm(ps_r[x], 1, 2 * x + 1, k)
            for x in range(2):
                for k in range(NK):
                    h_mm(ps_hn[x], 1, 8 + 2 * x + 1, k, start=(k == 0))
            for x in range(2):
                x_mm(ps_z[x], 1, 4 + 2 * x + 1)
                for k in range(NK):
                    h_mm(ps_z[x], 1, 4 + 2 * x + 1, k)

            # end-of-step h-free MMs into a never-read scratch bank: keep the
            # HAM clock-gate warm through the h' stall (cold MMs run at 1.2
            # instead of 2.4 GHz and cost ~1us/step otherwise).
            for w in range(10):
                nc.tensor.matmul(ps_warm[:, ts(w % 4, BC)], wgT[:, ts(w, 128)],
                                 inputT[:, ts(min(t + 1, T - 1), BC)],
                                 start=True, stop=True)

            # gate math: per-half chains with FORCED static per-engine order
            # (Tile's static schedule otherwise runs half-B sigmoids before
            # tanh_A, stalling the chain on half-B psums).
            h_nxt = hpool.tile([128, NK * BC], bf16, tag="h", name=f"h_{t}")
            act_prev = dve_prev = None

            def act_chain(bi):
                return bi

            def dve_chain(bi):
                return bi

            for x, js in enumerate(halves):
                hs = ds(2 * x * BC, 2 * BC)
                sr = tmp.tile([128, 2 * BC], f32, tag=f"sr{x}", name=f"sr{x}_{t}")
                t1 = tmp.tile([128, 2 * BC], f32, tag=f"t1{x}", name=f"t1{x}_{t}")
                # tail ops in bf16: SBUF-only bf16 DVE ops hit the 2-4x
                # perf mode, shortening the critical h' chain
                n_t = tmp.tile([128, 2 * BC], bf16, tag=f"n{x}", name=f"n{x}_{t}")
                t3 = tmp.tile([128, 2 * BC], bf16, tag=f"t3{x}", name=f"t3{x}_{t}")
                szc = tmp.tile([128, 2 * BC], bf16, tag=f"szc{x}", name=f"szc{x}_{t}")
                t4 = tmp.tile([128, 2 * BC], bf16, tag=f"t4{x}", name=f"t4{x}_{t}")
                act_chain(nc.scalar.activation(sr[:], ps_r[x][:], AF.Sigmoid))
                if HAS_BHN:
                    for i, j in enumerate(js):
                        dve_chain(nc.vector.tensor_scalar_add(
                            ps_hn[x][:, ts(i, BC)], ps_hn[x][:, ts(i, BC)],
                            consts[:, C_BHN + j : C_BHN + j + 1]))
                dve_chain(nc.vector.tensor_mul(t1[:], sr[:], ps_hn[x][:]))
                dve_chain(nc.vector.tensor_add(
                    t1[:], t1[:], ps_xn[:, ds(2 * x * BC, 2 * BC)]))
                act_chain(nc.scalar.activation(
                    szc[:], ps_z[x][:], AF.Sigmoid, scale=-1.0))
                act_chain(nc.scalar.activation(n_t[:], t1[:], AF.Tanh))
                # h' = h + (1-z)*(n - h)
                dve_chain(nc.vector.tensor_sub(t3[:], n_t[:], h_all[:, hs]))
                dve_chain(nc.vector.tensor_mul(t4[:], szc[:], t3[:]))
                dve_chain(nc.vector.tensor_add(h_nxt[:, hs], h_all[:, hs], t4[:]))
            h_all = h_nxt

        # ---- epilogue ----
        psum_ctx.close()
        epsum = ctx.enter_context(tc.tile_pool(name="epsum", bufs=1, space="PSUM"))
        ident16 = wpool.tile([128, 128], bf16)
        make_identity(nc, ident16[:])
        ident32 = wpool.tile([128, 128], f32)
        make_identity(nc, ident32[:])

        # hn_out = h_last.T  (batch-major)
        ps_tr = epsum.tile([128, H], bf16)
        for k in range(NK):
            nc.tensor.transpose(ps_tr[:, ts(k, 128)], h_all[:, ts(k, BC)], ident16[:])
        hn_sb = state.tile([128, H], f32)
        nc.vector.tensor_copy(hn_sb[:], ps_tr[:])
        nc.sync.dma_start(d_out_hn[:, :], hn_sb[:])

        # rh = relu(h), bf16, feature-major
        rh = state.tile([128, NK * BC], bf16)
        nc.scalar.activation(rh[:], h_all[:], AF.Relu)

        # out2 = relu(W3 @ rh + b3): [256 -> 2 tiles, BC]
        ps_o2 = epsum.tile([128, 2 * BC], f32)
        for mt in range(2):
            for k in range(NK):
                nc.tensor.matmul(
                    ps_o2[:, ts(mt, BC)],
                    w3T[:, ds(k * D2 + mt * 128, 128)],
                    rh[:, ts(k, BC)],
                    start=(k == 0),
                    stop=(k == NK - 1),
                )
        out2 = state.tile([128, 2 * BC], bf16)
        for mt in range(2):
            nc.scalar.activation(
                out2[:, ts(mt, BC)], ps_o2[:, ts(mt, BC)], AF.Relu,
                bias=consts[:, C_B3 + mt : C_B3 + mt + 1],
            )

        # P_all = Wh_flat @ out2 + bh: [512 (c,o) -> 4 tiles, BC]
        ps_ph = epsum.tile([128, 4 * BC], f32)
        for mt in range(4):
            for k in range(2):
                nc.tensor.matmul(
                    ps_ph[:, ts(mt, BC)],
                    whT[:, ds(k * 512 + mt * 128, 128)],
                    out2[:, ts(k, BC)],
                    start=(k == 0),
                    stop=(k == 1),
                )
        pall = state.tile([128, 4 * BC], f32)
        mask = state.tile([128, BC], f32)
        for mt in range(4):
            nc.vector.tensor_scalar_add(
                pall[:, ts(mt, BC)], ps_ph[:, ts(mt, BC)],
                consts[:, C_BH + mt : C_BH + mt + 1],
            )
            # mask[p, b] = (cult[b] == c_of_row_p)
            nc.vector.tensor_scalar(
                mask[:], cultb[:],
                consts[:, C_IOTA + mt : C_IOTA + mt + 1], None,
                op0=mybir.AluOpType.is_equal,
            )
            nc.vector.tensor_mul(pall[:, ts(mt, BC)], pall[:, ts(mt, BC)], mask[:])

        # params[o, b] = sum_c pall[(c,o), b]  via selector matmul
        ps_pp = epsum.tile([128, BC], f32)
        for k in range(4):
            nc.tensor.matmul(
                ps_pp[0:D_OUT, :], selT[:, ts(k, D_OUT)], pall[:, ts(k, BC)],
                start=(k == 0), stop=(k == 3),
            )
        pp_sb = state.tile([D_OUT, BC], f32)
        nc.vector.tensor_copy(pp_sb[:], ps_pp[0:D_OUT, :])
        # transpose to [BC, 16]
        ps_ptr = epsum.tile([128, D_OUT], f32)
        nc.tensor.transpose(ps_ptr[:, :], pp_sb[:], ident32[0:D_OUT, 0:D_OUT])
        p_sb = state.tile([BC, D_OUT], f32)
        nc.vector.tensor_copy(p_sb[:], ps_ptr[:, :])
        nc.sync.dma_start(d_out_p[:, :], p_sb[:])

    nc.compile()
    return nc


def _prep_host(inputs):
    """Fold weights and build per-core input maps."""
    inp = np.asarray(inputs["input"], np.float32)
    hn = np.asarray(inputs["hn"], np.float32)
    cult = np.asarray(inputs["cultivars"], np.int32)
    W1 = np.asarray(inputs["W1"], np.float64)
    b1 = np.asarray(inputs["b1"], np.float64)
    W2 = np.asarray(inputs["W2"], np.float64)
    b2 = np.asarray(inputs["b2"], np.float64)
    W_ih = np.asarray(inputs["W_ih"], np.float64)
    W_hh = np.asarray(inputs["W_hh"], np.float32)
    b_ih = np.asarray(inputs["b_ih"], np.float64)
    b_hh = np.asarray(inputs["b_hh"], np.float32)
    W3 = np.asarray(inputs["W3"], np.float32)
    b3 = np.asarray(inputs["b3"], np.float32)
    Wh = np.asarray(inputs["Wh"], np.float32)
    bh = np.asarray(inputs["bh"], np.float32)

    Wg = (W_ih @ (W2 @ W1)).astype(np.float32)          # [1536, 32]
    bg = (W_ih @ (W2 @ b1 + b2) + b_ih).astype(np.float32)  # [1536]

    def fm(x, nk):  # [K, M] -> sbuf [128, nk*M] feature-major K-chunked
        K, M = x.shape
        return np.ascontiguousarray(
            x.reshape(nk, 128, M).transpose(1, 0, 2).reshape(128, nk * M)
        )

    bias_row = np.concatenate([
        bg[:1024] + b_hh[:1024].astype(np.float32),   # r,z biases
        bg[1024:],                                    # xn bias
    ])[None, :]                                       # [1, 1536]
    wgT = np.ascontiguousarray(np.concatenate(
        [Wg.T, bias_row, np.zeros((128 - D_IN - 1, G3), np.float32)], axis=0
    )).astype(BF16)                                   # [128, 1536] zero-padded
    whhT = fm(W_hh.T.astype(np.float32), NK).astype(BF16)              # [128, 4*1536]
    w3T = fm(W3.T, NK).astype(BF16)                                    # [128, 4*256]
    Wh_flat = Wh.reshape(N_CULT * D_OUT, D2)                           # [512, 256]
    whT = fm(np.ascontiguousarray(Wh_flat.T), 2).astype(BF16)          # [128, 2*512]
    sel = (np.arange(512)[:, None] % 16 == np.arange(16)[None, :]).astype(np.float32)
    selT = fm(sel, 4)                                                  # [128, 64]

    consts = np.zeros((128, 26), np.float32)
    brz = bg[:1024] + b_hh[:1024].astype(np.float32)
    consts[:, 0:4] = brz[:512].reshape(4, 128).T
    consts[:, 4:8] = -brz[512:1024].reshape(4, 128).T
    consts[:, 8:12] = bg[1024:].reshape(4, 128).T
    consts[:, 12:16] = b_hh[1024:].reshape(4, 128).T
    consts[:, 16:18] = b3.reshape(2, 128).T
    consts[:, 18:22] = bh.reshape(512)[:, None].reshape(4, 128).T
    consts[:, 22:26] = ((np.arange(512) // 16).astype(np.float32)).reshape(4, 128).T

    shared = {
        "wgT": wgT, "whhT": whhT, "w3T": w3T, "whT": whT,
        "selT": selT, "consts": consts,
    }

    in_maps = []
    for c in range(NCORES):
        b0 = c * BC
        ic = inp[b0 : b0 + BC]                       # [BC, T, D]
        inputT = np.ascontiguousarray(np.concatenate([
            ic.transpose(2, 1, 0).reshape(D_IN, T * BC),
            np.ones((1, T * BC), np.float32),
            np.zeros((128 - D_IN - 1, T * BC), np.float32),
        ], axis=0)).astype(BF16)                      # [128, T*BC], free=(t,b)
        h0 = hn[0, b0 : b0 + BC]                      # [BC, H]
        hnT = np.ascontiguousarray(
            h0.T.reshape(NK, 128, BC).transpose(1, 0, 2).reshape(128, H)
        )
        cultf = cult[b0 : b0 + BC, 0].astype(np.float32)[None, :]
        in_maps.append({**shared, "inputT": inputT, "hnT": hnT, "cultf": cultf})
    return in_maps


def _ensure_ntff_hook():
    """Inject antenv.axon_hooks (absent in this image) so trace=True works."""
    import sys
    import types

    try:
        from antenv.axon_hooks import get_axon_ntff_profile_hook  # noqa: F401
        return
    except ImportError:
        pass
    try:
        import antenv
        from trn_agent_boot.trn_boot import _ntff_profile_via_ctypes

        hook = _ntff_profile_via_ctypes("/opt/axon/libaxon_pjrt.so")
        mod = types.ModuleType("antenv.axon_hooks")
        mod._hook = hook
        mod.set_axon_ntff_profile_hook = lambda h: setattr(mod, "_hook", h)
        mod.get_axon_ntff_profile_hook = lambda: mod._hook
        sys.modules["antenv.axon_hooks"] = mod
        antenv.axon_hooks = mod
    except Exception as e:  # degrade to no-trace
        print(f"ntff hook setup failed: {e}")


def kernel(**inputs):
    from concourse.bass_utils import run_bass_kernel_spmd

    if bool(int(os.environ.get("KERNEL_TRACE", "0"))):
        _ensure_ntff_hook()
    has_bhn = bool(np.any(np.asarray(inputs["b_hh"])[1024:] != 0))
    if _CACHE.get("nc") is None or _CACHE.get("has_bhn") != has_bhn:
        _CACHE["has_bhn"] = has_bhn
        _CACHE["nc"] = _build_nc()
    nc = _CACHE["nc"]

    in_maps = _prep_host(inputs)
    res = run_bass_kernel_spmd(
        nc, in_maps, core_ids=list(range(NCORES)),
        trace=bool(int(os.environ.get("KERNEL_TRACE", "0"))),
    )
    if res.exec_time_ns is not None:
        _CACHE["exec_time_ns"] = res.exec_time_ns
        _CACHE["results"] = res

    params = np.concatenate([r["out_p"] for r in res.results], axis=0)
    hn_out = np.concatenate([r["out_hn"] for r in res.results], axis=0)[None]
    return (params.astype(np.float32), hn_out.astype(np.float32))
